# revision 38
# baseline (speedup 1.0000x reference)
"""Multi-head attention (B=4, N=2048, C=1024, H=16) on 8 TRN2 NeuronCores.

Sharding: core c = (batch b = c//2, head-group hg = c%2), 8 heads per group.
Each core computes its head-group's attention for its batch plus the partial
output projection against the matching w_out rows; the host sums the two
partials per batch and adds the bias terms (exact: softmax rows sum to 1, so
the v-bias contributes b_v @ w_out + b_out as a constant row).

Device pipeline (per core), all matmuls bf16 (inputs pre-cast on host):
  1. v token-major with a fused ones column per head (the ones column makes
     the PV matmul accumulate the softmax denominator in psum row 64 free)
  2. heads processed in PAIRS (A at partitions 0-63, B at 64-127 of the same
     qkT tile): per 2-step block one [128,1024] psum score tile per step is
     filled by two K=64 matmuls on PE row-tiles T0/T8 (64x128 tiling mode),
     one ScalarE exp covers both heads, then two K=128 PV matmuls per step
     accumulate po_A/po_B. Normalization is deferred off the PE critical
     path: denominator rows gather via DMA into one [8,512] tile, a single
     batched DVE reciprocal, gpsimd partition_broadcast, DVE multiply —
     emitted into the NEXT pair's instruction stream so the in-order PE
     queue never stalls on the DVE chain. The final pair normalizes per
     query quarter and folds the output projection into its stream.
  3. ~40 junk warm-up matmuls run during the startup DMA wait to open the
     HAM clock gate (idle PE defaults to 1.2 GHz half clock).
"""

import numpy as np

B, N, C = 4, 2048, 1024
H, Dh = 16, 64
HG = 8  # heads per core
P = 128
KK = C // P       # 8 contraction tiles for the projections
NT = N // P       # 16 nk tiles
NQ4 = 4           # 512-token query chunks

_CACHE = {}


def _build():
    import concourse.bass as bass
    import concourse.tile as tile
    from concourse import mybir, bacc
    from contextlib import ExitStack

    f32 = mybir.dt.float32
    f32r = mybir.dt.float32r
    bf16 = mybir.dt.bfloat16
    FT = mybir.ActivationFunctionType
    OP = mybir.AluOpType

    nc = bacc.Bacc("TRN2", target_bir_lowering=False, debug=False)

    xT = nc.dram_tensor("xT", [C, N], bf16, kind="ExternalInput").ap()
    wq = nc.dram_tensor("wq", [C, 512], bf16, kind="ExternalInput").ap()
    wk = nc.dram_tensor("wk", [C, 512], bf16, kind="ExternalInput").ap()
    wv = nc.dram_tensor("wv", [C, 512], bf16, kind="ExternalInput").ap()
    bqk = nc.dram_tensor("bqk", [P, 8], f32, kind="ExternalInput").ap()
    wo = nc.dram_tensor("wo", [512, C], bf16, kind="ExternalInput").ap()
    out = nc.dram_tensor("out", [N, C], f32, kind="ExternalOutput").ap()

    with tile.TileContext(nc) as tc, ExitStack() as ctx, \
         nc.allow_low_precision(reason="bf16 attention pipeline"):
        pool = lambda name, bufs: ctx.enter_context(
            tc.tile_pool(name=name, bufs=bufs))
        qkT_pool = pool("qkT", 1)
        v_pool = pool("v", 1)
        attT_pool = pool("attT", 1)
        const_pool = pool("const", 1)
        x_pool = pool("x", 1)
        exp_pool = pool("expst", 10)
        ou_pool = pool("ou", 8)
        rp_pool = pool("rp", 2)
        out_pool = pool("outst", 2)
        pscore = ctx.enter_context(
            tc.tile_pool(name="pscore", bufs=2, space="PSUM"))
        ppo = ctx.enter_context(tc.tile_pool(name="ppo", bufs=2, space="PSUM"))
        pfill = ctx.enter_context(
            tc.tile_pool(name="pfill", bufs=2, space="PSUM"))

        qkT = [qkT_pool.tile([P, N], bf16, tag=f"qkT{i}", name=f"qkT{i}")
               for i in range(8)]
        vt = [v_pool.tile([P, HG * 65], bf16, tag=f"v{i}", name=f"vt{i}")
              for i in range(NT)]
        attT = [attT_pool.tile([P, N], bf16, tag=f"attT{i}", name=f"attT{i}")
                for i in range(4)]

        biasqk_raw = const_pool.tile([P, 8], f32, tag="bqkr", name="biasqk_raw")
        nc.gpsimd.dma_start(biasqk_raw[:], bqk)
        biasqk = const_pool.tile([P, 8], f32, tag="bqk", name="biasqk")
        nc.vector.tensor_copy(biasqk[:], biasqk_raw[:])

        ET = mybir.EngineType

        # HAM warm-up: keep the PE array busy on junk matmuls during the
        # startup DMA wait so the clock gate opens before real work lands
        # (idle PE defaults to K=4/8 half clock; ~3.4us of sustained
        # activity un-throttles it)
        wup = const_pool.tile([P, 512], bf16, tag="wup", name="wup")
        nc.vector.memset(wup[:], 0.0)
        wups = pfill.tile([P, 512], f32, tag="pf", name="psa")
        for _ in range(40):
            nc.tensor.matmul(wups[:], wup[:, 0:P], wup[:],
                             start=True, stop=True)

        def load(ap, name, eng=None):
            return x_pool.tile_from(ap, name=name, forced_dma_engine=eng)

        # spread the startup loads over several DMA queues so the first
        # projection groups are not serialized behind one 7MB stream
        # x dominates the startup critical path: split it over both fast
        # HWDGE queues; q/k weights go on the SWDGE queue in parallel,
        # late-need wv/wo behind x
        xt = [load(xT[kk * P:(kk + 1) * P, :], f"xt{kk}",
                   ET.SP if kk % 2 == 0 else ET.Activation)
              for kk in range(KK)]
        wqk_t = [load(wq[kk * P:(kk + 1) * P, :], f"wqt{kk}", ET.Pool)
                 for kk in range(KK)]
        wqk_t += [load(wk[kk * P:(kk + 1) * P, :], f"wkt{kk}", ET.Pool)
                  for kk in range(KK)]
        wv_t = [load(wv[kk * P:(kk + 1) * P, :], f"wvt{kk}", ET.SP)
                for kk in range(KK)]
        wo_t = [load(wo[kk * P:(kk + 1) * P, :], f"wot{kk}", ET.Activation)
                for kk in range(4)]

        def qk_group(mt, j):
            # q (mt 0-3) / k (mt 4-7) projection: heads 2*(mt%4), 2*(mt%4)+1
            ps = pfill.tile([P, 512], f32, tag="pf", name="psa")
            for kk in range(KK):
                w_ap = wqk_t[(mt // 4) * KK + kk][:, (mt % 4) * P:
                                                  (mt % 4 + 1) * P]
                nc.tensor.matmul(ps[:], w_ap,
                                 xt[kk][:, j * 512:(j + 1) * 512],
                                 start=(kk == 0), stop=(kk == KK - 1))
            nc.vector.tensor_scalar_add(
                qkT[mt][:, j * 512:(j + 1) * 512], ps[:],
                biasqk[:, mt:mt + 1])

        def v_group(mg):
            ps = pfill.tile([P, 512], f32, tag="pf", name="psa")
            for kk in range(KK):
                nc.tensor.matmul(ps[:], xt[kk][:, mg * P:(mg + 1) * P],
                                 wv_t[kk][:],
                                 start=(kk == 0), stop=(kk == KK - 1))
            vg = vt[mg][:].rearrange("p (h c) -> p h c", c=65)
            nc.vector.tensor_copy(vg[:, :, 0:64],
                                  ps[:].rearrange("p (h c) -> p h c", c=64))
            nc.vector.memset(vg[:, :, 64:65], 1.0)

        def out_proj(m):
            ob = out_pool.tile([P, C], f32, tag="ob", name="ob")
            for c in range(2):
                ps = pfill.tile([P, 512], f32, tag="pf", name="psa")
                for kk in range(4):
                    nc.tensor.matmul(ps[:],
                                     attT[kk][:, m * P:(m + 1) * P],
                                     wo_t[kk][:, c * 512:(c + 1) * 512],
                                     start=(kk == 0), stop=(kk == 3))
                nc.vector.tensor_copy(ob[:, c * 512:(c + 1) * 512], ps[:])
            nc.sync.dma_start(out[m * P:(m + 1) * P, :], ob[:])

        def attention_pair(g, fillers, deadlines=None, inline_v=False,
                           final=False, prework=None):
            hA, hB = 2 * g, 2 * g + 1
            qA = qkT[g][0:64, :]
            qB = qkT[g][64:128, :]
            kA = qkT[4 + g][0:64, :]
            kB = qkT[4 + g][64:128, :]
            nfill = len(fillers)
            fi = 0
            D = 4  # scores/exp run D steps ahead of PV
            es = {}
            po = {}
            otiles = []  # (off, q, o) accumulated over quarters
            # PE-side work deferred so it never stalls the in-order PE
            # queue behind a DVE chain: items carry a ready-block and are
            # emitted at most one per block once ready
            pending = [(4, w) for w in (prework or [])]

            def make_finish(tiles, rr, nsel):
                def fin():
                    for i, (off, q, o) in enumerate(tiles):
                        # broadcast 1/den row across 64 partitions on the
                        # (otherwise idle) gpsimd engine instead of a PE
                        # ones-matmul: keeps the tensor engine and PSUM out
                        # of the normalize chain entirely
                        rbc = rp_pool.tile([1, 512], f32, tag="rbc",
                                           name="rbct")
                        nc.sync.dma_start(rbc[:], rr[i:i + 1, :])
                        pbs = rp_pool.tile([64, 512], f32, tag="pbs",
                                           name="pbst")
                        nc.gpsimd.partition_broadcast(pbs[:], rbc[:])
                        nc.vector.tensor_tensor(
                            attT[g][off:off + 64, q * 512:(q + 1) * 512],
                            o[0:64, :], pbs[:], op=OP.mult)
                return fin

            total = NQ4 * NT  # 64 steps
            # 2-step mode blocks: [scores s, s+1] (64x128 tiling mode),
            # then [PV s-D, s-D+1] (128x128) — halves PE mode switches
            for b in range(0, total + D, 2):
                for s in (b, b + 1):
                    if s >= total:
                        continue
                    q, t = divmod(s, NT)
                    ps = pscore.tile([P, 1024], f32, tag="sc", name="psc")
                    nc.tensor.matmul(ps[:, 0:512],
                                     kA[:, t * P:(t + 1) * P],
                                     qA[:, q * 512:(q + 1) * 512],
                                     start=True, stop=True)
                    nc.tensor.matmul(ps[:, 512:1024],
                                     kB[:, t * P:(t + 1) * P],
                                     qB[:, q * 512:(q + 1) * 512],
                                     start=True, stop=True)
                    e = exp_pool.tile([P, 1024], bf16, tag="e", name="et")
                    nc.scalar.activation(e[:], ps[:], FT.Exp, scale=Dh ** -0.5)
                    es[s] = e
                for s in (b, b + 1):
                    s2 = s - D
                    if s2 < 0 or s2 >= total:
                        continue
                    q2, t2 = divmod(s2, NT)
                    if t2 == 0:
                        po["A"] = ppo.tile([65, 512], f32, tag="po",
                                           name="poA")
                        po["B"] = ppo.tile([65, 512], f32, tag="po",
                                           name="poB")
                    e2 = es.pop(s2)
                    if inline_v and q2 == 0:
                        v_group(t2)
                    nc.tensor.matmul(po["A"][:],
                                     vt[t2][:, hA * 65:hA * 65 + 65],
                                     e2[:, 0:512],
                                     start=(t2 == 0), stop=(t2 == NT - 1))
                    nc.tensor.matmul(po["B"][:],
                                     vt[t2][:, hB * 65:hB * 65 + 65],
                                     e2[:, 512:1024],
                                     start=(t2 == 0), stop=(t2 == NT - 1))
                    if t2 == NT - 1:
                        for X, off in (("A", 0), ("B", 64)):
                            p = po.pop(X)
                            o = ou_pool.tile([65, 512], f32, tag="o",
                                             name="otile")
                            # split the two PSUM evacuations across
                            # engines so the ppo ring frees faster
                            if X == "A":
                                nc.scalar.copy(o[:], p[:])
                            else:
                                nc.vector.tensor_copy(o[:], p[:])
                            otiles.append((off, q2, o))
                        if final:
                            # per-quarter normalize: DVE-side (gather +
                            # reciprocal) now, PE-side (broadcast + mult)
                            # and the quarter's output projection deferred
                            # into following blocks
                            dd = rp_pool.tile([2, 512], f32, tag="dd2",
                                              name="dd2t")
                            for i, (off, q, o) in enumerate(otiles):
                                nc.sync.dma_start(dd[i:i + 1, :],
                                                  o[64:65, :])
                            rr = rp_pool.tile([2, 512], f32, tag="rr2",
                                              name="rr2t")
                            with nc.allow_low_precision(
                                    reason="softmax denom"):
                                nc.vector.reciprocal(rr[:], dd[:])
                            blk = b // 2
                            pending.append(
                                (blk + 1, make_finish(otiles, rr, 2)))
                            for mi, m in enumerate(range(4 * q2,
                                                         4 * q2 + 4)):
                                pending.append(
                                    (blk + 3 + mi, lambda m=m: out_proj(m)))
                            otiles = []
                if pending and pending[0][0] <= b // 2:
                    pending.pop(0)[1]()
                while fi < nfill and (
                        fi < ((b + 2) * nfill) // (total + D)
                        or (deadlines and fi < len(deadlines)
                            and deadlines[fi] <= b // 2)):
                    fillers[fi]()
                    fi += 1

            for _, w in pending:
                w()
            if final:
                return None
            # deferred normalize, emitted into the next pair's stream:
            # DVE-side now (gather den rows to partitions 0-7 via DMA —
            # engine APs cannot start at unaligned partitions, DMA can
            # place anywhere — then one batched reciprocal), PE-side
            # returned as a closure
            dd = rp_pool.tile([8, 512], f32, tag="dd", name="ddt")
            for i, (off, q, o) in enumerate(otiles):
                nc.sync.dma_start(dd[i:i + 1, :], o[64:65, :])
            rr = rp_pool.tile([8, 512], f32, tag="rr", name="rrt")
            with nc.allow_low_precision(reason="softmax denom"):
                nc.vector.reciprocal(rr[:], dd[:])
            return make_finish(otiles, rr, 8)

        # prologue: q chunk 0 + all of k for pair 0 (k columns are consumed
        # across all chunks within the first query quarter); remaining
        # pair-0 q chunks become deadline-paced fillers, v inlined into
        # pair 0, later pairs' projections spread as fillers, and the
        # output projection folds into the final pair per query quarter
        qk_group(0, 0)
        for j in range(NQ4):
            qk_group(4, j)
        finish_prev = None
        for g in range(4):
            fillers, deadlines = [], []
            if g == 0:
                for j in range(1, NQ4):
                    fillers.append(lambda j=j: qk_group(0, j))
                    deadlines.append(8 * j - 3)
            if g < 3:
                for mt in (4 + g + 1, g + 1):
                    for j in range(NQ4):
                        fillers.append(lambda mt=mt, j=j: qk_group(mt, j))
                        deadlines.append(10 ** 9)
            prework = [finish_prev] if finish_prev else []
            finish_prev = attention_pair(g, fillers, deadlines,
                                         inline_v=(g == 0), final=(g == 3),
                                         prework=prework)

    nc.compile()
    return nc


def _in_maps(x, w_qkv, b_qkv, w_out):
    import ml_dtypes
    bf = ml_dtypes.bfloat16
    x = np.asarray(x, np.float32)
    w_qkv = np.asarray(w_qkv, np.float32)
    b_qkv = np.asarray(b_qkv, np.float32)
    w_out = np.asarray(w_out, np.float32)
    maps = []
    for core in range(8):
        b, hg = core // 2, core % 2
        s = slice(hg * 512, hg * 512 + 512)
        maps.append({
            "xT": np.ascontiguousarray(x[b].T).astype(bf),
            "wq": np.ascontiguousarray(w_qkv[:, 0 * C:1 * C][:, s]).astype(bf),
            "wk": np.ascontiguousarray(w_qkv[:, 1 * C:2 * C][:, s]).astype(bf),
            "wv": np.ascontiguousarray(w_qkv[:, 2 * C:3 * C][:, s]).astype(bf),
            "bqk": np.ascontiguousarray(np.concatenate(
                [b_qkv[0 * C:1 * C][s], b_qkv[1 * C:2 * C][s]])
                .reshape(8, P).T),
            "wo": np.ascontiguousarray(w_out[s, :]).astype(bf),
        })
    return maps


def _gather(results, b_qkv, b_out, w_out):
    out = np.zeros((B, N, C), np.float32)
    for core in range(8):
        out[core // 2] += np.asarray(results[core]["out"], np.float32)
    # exact bias terms: softmax rows sum to 1, so +b_v contributes b_v @ w_out
    out += (np.asarray(b_qkv[2 * C:3 * C], np.float32)
            @ np.asarray(w_out, np.float32) + np.asarray(b_out, np.float32))
    return out


def run(x, w_qkv, b_qkv, w_out, b_out, trace=False):
    from concourse.bass_utils import run_bass_kernel_spmd
    if "nc" not in _CACHE:
        _CACHE["nc"] = _build()
    res = run_bass_kernel_spmd(_CACHE["nc"], _in_maps(x, w_qkv, b_qkv, w_out),
                               list(range(8)), trace=trace)
    _CACHE["last_res"] = res
    return _gather(res.results, b_qkv, b_out, w_out), res.exec_time_ns


def kernel(x, w_qkv, b_qkv, w_out, b_out):
    out, _ = run(x, w_qkv, b_qkv, w_out, b_out)
    return out


# revision 39
# speedup vs baseline: 1.0179x; 1.0179x over previous
"""Multi-head attention (B=4, N=2048, C=1024, H=16) on 8 TRN2 NeuronCores.

Sharding: core c = (batch b = c//2, head-group hg = c%2), 8 heads per group.
Each core computes its head-group's attention for its batch plus the partial
output projection against the matching w_out rows; the host sums the two
partials per batch and adds the bias terms (exact: softmax rows sum to 1, so
the v-bias contributes b_v @ w_out + b_out as a constant row).

Device pipeline (per core), all matmuls bf16 (inputs pre-cast on host):
  1. v token-major with a fused ones column per head (the ones column makes
     the PV matmul accumulate the softmax denominator in psum row 64 free)
  2. heads processed in PAIRS (A at partitions 0-63, B at 64-127 of the same
     qkT tile): per 2-step block one [128,1024] psum score tile per step is
     filled by two K=64 matmuls on PE row-tiles T0/T8 (64x128 tiling mode),
     one ScalarE exp covers both heads, then two K=128 PV matmuls per step
     accumulate po_A/po_B. Normalization is deferred off the PE critical
     path: denominator rows gather via DMA into one [8,512] tile, a single
     batched DVE reciprocal, gpsimd partition_broadcast, DVE multiply —
     emitted into the NEXT pair's instruction stream so the in-order PE
     queue never stalls on the DVE chain. The final pair normalizes per
     query quarter and folds the output projection into its stream.
  3. ~40 junk warm-up matmuls run during the startup DMA wait to open the
     HAM clock gate (idle PE defaults to 1.2 GHz half clock).
"""

import numpy as np

B, N, C = 4, 2048, 1024
H, Dh = 16, 64
HG = 8  # heads per core
P = 128
KK = C // P       # 8 contraction tiles for the projections
NT = N // P       # 16 nk tiles
NQ4 = 4           # 512-token query chunks

_CACHE = {}


def _build():
    import concourse.bass as bass
    import concourse.tile as tile
    from concourse import mybir, bacc
    from contextlib import ExitStack

    f32 = mybir.dt.float32
    f32r = mybir.dt.float32r
    bf16 = mybir.dt.bfloat16
    FT = mybir.ActivationFunctionType
    OP = mybir.AluOpType

    nc = bacc.Bacc("TRN2", target_bir_lowering=False, debug=False)

    xT = nc.dram_tensor("xT", [C, N], bf16, kind="ExternalInput").ap()
    wq = nc.dram_tensor("wq", [C, 512], bf16, kind="ExternalInput").ap()
    wk = nc.dram_tensor("wk", [C, 512], bf16, kind="ExternalInput").ap()
    wv = nc.dram_tensor("wv", [C, 512], bf16, kind="ExternalInput").ap()
    bqk = nc.dram_tensor("bqk", [P, 8], f32, kind="ExternalInput").ap()
    wo = nc.dram_tensor("wo", [512, C], bf16, kind="ExternalInput").ap()
    out = nc.dram_tensor("out", [N, C], f32, kind="ExternalOutput").ap()

    with tile.TileContext(nc) as tc, ExitStack() as ctx, \
         nc.allow_low_precision(reason="bf16 attention pipeline"):
        pool = lambda name, bufs: ctx.enter_context(
            tc.tile_pool(name=name, bufs=bufs))
        qkT_pool = pool("qkT", 1)
        v_pool = pool("v", 1)
        attT_pool = pool("attT", 1)
        const_pool = pool("const", 1)
        x_pool = pool("x", 1)
        exp_pool = pool("expst", 10)
        ou_pool = pool("ou", 8)
        rp_pool = pool("rp", 2)
        out_pool = pool("outst", 2)
        pscore = ctx.enter_context(
            tc.tile_pool(name="pscore", bufs=2, space="PSUM"))
        ppo = ctx.enter_context(tc.tile_pool(name="ppo", bufs=2, space="PSUM"))
        pfill = ctx.enter_context(
            tc.tile_pool(name="pfill", bufs=2, space="PSUM"))

        qkT = [qkT_pool.tile([P, N], bf16, tag=f"qkT{i}", name=f"qkT{i}")
               for i in range(8)]
        vt = [v_pool.tile([P, HG * 65], bf16, tag=f"v{i}", name=f"vt{i}")
              for i in range(NT)]
        attT = [attT_pool.tile([P, N], bf16, tag=f"attT{i}", name=f"attT{i}")
                for i in range(4)]

        biasqk_raw = const_pool.tile([P, 8], f32, tag="bqkr", name="biasqk_raw")
        nc.gpsimd.dma_start(biasqk_raw[:], bqk)
        biasqk = const_pool.tile([P, 8], f32, tag="bqk", name="biasqk")
        nc.vector.tensor_copy(biasqk[:], biasqk_raw[:])

        ET = mybir.EngineType

        # HAM warm-up: keep the PE array busy on junk matmuls during the
        # startup DMA wait so the clock gate opens before real work lands
        # (idle PE defaults to K=4/8 half clock; ~3.4us of sustained
        # activity un-throttles it)
        wup = const_pool.tile([P, 512], bf16, tag="wup", name="wup")
        nc.vector.memset(wup[:], 0.0)
        wups = pfill.tile([P, 512], f32, tag="pf", name="psa")
        for _ in range(40):
            nc.tensor.matmul(wups[:], wup[:, 0:P], wup[:],
                             start=True, stop=True)

        def load(ap, name, eng=None):
            return x_pool.tile_from(ap, name=name, forced_dma_engine=eng)

        # spread the startup loads over several DMA queues so the first
        # projection groups are not serialized behind one 7MB stream
        # x dominates the startup critical path: split it over both fast
        # HWDGE queues; q/k weights go on the SWDGE queue in parallel,
        # late-need wv/wo behind x
        xt = [load(xT[kk * P:(kk + 1) * P, :], f"xt{kk}",
                   ET.SP if kk % 2 == 0 else ET.Activation)
              for kk in range(KK)]
        wqk_t = [load(wq[kk * P:(kk + 1) * P, :], f"wqt{kk}", ET.Pool)
                 for kk in range(KK)]
        wqk_t += [load(wk[kk * P:(kk + 1) * P, :], f"wkt{kk}", ET.Pool)
                  for kk in range(KK)]
        wv_t = [load(wv[kk * P:(kk + 1) * P, :], f"wvt{kk}", ET.SP)
                for kk in range(KK)]
        wo_t = [load(wo[kk * P:(kk + 1) * P, :], f"wot{kk}", ET.Activation)
                for kk in range(4)]

        def qk_group(mt, j):
            # q (mt 0-3) / k (mt 4-7) projection: heads 2*(mt%4), 2*(mt%4)+1
            ps = pfill.tile([P, 512], f32, tag="pf", name="psa")
            for kk in range(KK):
                w_ap = wqk_t[(mt // 4) * KK + kk][:, (mt % 4) * P:
                                                  (mt % 4 + 1) * P]
                nc.tensor.matmul(ps[:], w_ap,
                                 xt[kk][:, j * 512:(j + 1) * 512],
                                 start=(kk == 0), stop=(kk == KK - 1))
            nc.vector.tensor_scalar_add(
                qkT[mt][:, j * 512:(j + 1) * 512], ps[:],
                biasqk[:, mt:mt + 1])

        def v_group(mg):
            ps = pfill.tile([P, 512], f32, tag="pf", name="psa")
            for kk in range(KK):
                nc.tensor.matmul(ps[:], xt[kk][:, mg * P:(mg + 1) * P],
                                 wv_t[kk][:],
                                 start=(kk == 0), stop=(kk == KK - 1))
            vg = vt[mg][:].rearrange("p (h c) -> p h c", c=65)
            nc.vector.tensor_copy(vg[:, :, 0:64],
                                  ps[:].rearrange("p (h c) -> p h c", c=64))
            nc.vector.memset(vg[:, :, 64:65], 1.0)

        def out_proj(m):
            ob = out_pool.tile([P, C], f32, tag="ob", name="ob")
            for c in range(2):
                ps = pfill.tile([P, 512], f32, tag="pf", name="psa")
                for kk in range(4):
                    nc.tensor.matmul(ps[:],
                                     attT[kk][:, m * P:(m + 1) * P],
                                     wo_t[kk][:, c * 512:(c + 1) * 512],
                                     start=(kk == 0), stop=(kk == 3))
                nc.vector.tensor_copy(ob[:, c * 512:(c + 1) * 512], ps[:])
            nc.sync.dma_start(out[m * P:(m + 1) * P, :], ob[:])

        def attention_pair(g, fillers, deadlines=None, inline_v=False,
                           final=False, prework=None):
            hA, hB = 2 * g, 2 * g + 1
            qA = qkT[g][0:64, :]
            qB = qkT[g][64:128, :]
            kA = qkT[4 + g][0:64, :]
            kB = qkT[4 + g][64:128, :]
            nfill = len(fillers)
            fi = 0
            D = 4  # scores/exp run D steps ahead of PV
            es = {}
            po = {}
            otiles = []  # (off, q, o) accumulated over quarters
            # PE-side work deferred so it never stalls the in-order PE
            # queue behind a DVE chain: items carry a ready-block and are
            # emitted at most one per block once ready
            pending = [(4, w) for w in (prework or [])]

            def make_finish(tiles, rr, nsel):
                def fin():
                    for i, (off, q, o) in enumerate(tiles):
                        # broadcast 1/den row across 64 partitions on the
                        # (otherwise idle) gpsimd engine instead of a PE
                        # ones-matmul: keeps the tensor engine and PSUM out
                        # of the normalize chain entirely
                        rbc = rp_pool.tile([1, 512], f32, tag="rbc",
                                           name="rbct")
                        nc.sync.dma_start(rbc[:], rr[i:i + 1, :])
                        pbs = rp_pool.tile([64, 512], f32, tag="pbs",
                                           name="pbst")
                        nc.gpsimd.partition_broadcast(pbs[:], rbc[:])
                        nc.vector.tensor_tensor(
                            attT[g][off:off + 64, q * 512:(q + 1) * 512],
                            o[0:64, :], pbs[:], op=OP.mult)
                return fin

            total = NQ4 * NT  # 64 steps
            # 2-step mode blocks: [scores s, s+1] (64x128 tiling mode),
            # then [PV s-D, s-D+1] (128x128) — halves PE mode switches
            for b in range(0, total + D, 2):
                for s in (b, b + 1):
                    if s >= total:
                        continue
                    q, t = divmod(s, NT)
                    ps = pscore.tile([P, 1024], f32, tag="sc", name="psc")
                    nc.tensor.matmul(ps[:, 0:512],
                                     kA[:, t * P:(t + 1) * P],
                                     qA[:, q * 512:(q + 1) * 512],
                                     start=True, stop=True)
                    nc.tensor.matmul(ps[:, 512:1024],
                                     kB[:, t * P:(t + 1) * P],
                                     qB[:, q * 512:(q + 1) * 512],
                                     start=True, stop=True)
                    e = exp_pool.tile([P, 1024], bf16, tag="e", name="et")
                    nc.scalar.activation(e[:], ps[:], FT.Exp, scale=Dh ** -0.5)
                    es[s] = e
                for s in (b, b + 1):
                    s2 = s - D
                    if s2 < 0 or s2 >= total:
                        continue
                    q2, t2 = divmod(s2, NT)
                    if t2 == 0:
                        po["A"] = ppo.tile([65, 512], f32, tag="po",
                                           name="poA")
                        po["B"] = ppo.tile([65, 512], f32, tag="po",
                                           name="poB")
                    e2 = es.pop(s2)
                    if inline_v and q2 == 0:
                        v_group(t2)
                    nc.tensor.matmul(po["A"][:],
                                     vt[t2][:, hA * 65:hA * 65 + 65],
                                     e2[:, 0:512],
                                     start=(t2 == 0), stop=(t2 == NT - 1))
                    nc.tensor.matmul(po["B"][:],
                                     vt[t2][:, hB * 65:hB * 65 + 65],
                                     e2[:, 512:1024],
                                     start=(t2 == 0), stop=(t2 == NT - 1))
                    if t2 == NT - 1:
                        for X, off in (("A", 0), ("B", 64)):
                            p = po.pop(X)
                            o = ou_pool.tile([65, 512], f32, tag="o",
                                             name="otile")
                            nc.vector.tensor_copy(o[:], p[:])
                            otiles.append((off, q2, o))
                        if final:
                            # per-quarter normalize: DVE-side (gather +
                            # reciprocal) now, PE-side (broadcast + mult)
                            # and the quarter's output projection deferred
                            # into following blocks
                            dd = rp_pool.tile([2, 512], f32, tag="dd2",
                                              name="dd2t")
                            for i, (off, q, o) in enumerate(otiles):
                                nc.sync.dma_start(dd[i:i + 1, :],
                                                  o[64:65, :])
                            rr = rp_pool.tile([2, 512], f32, tag="rr2",
                                              name="rr2t")
                            with nc.allow_low_precision(
                                    reason="softmax denom"):
                                nc.vector.reciprocal(rr[:], dd[:])
                            blk = b // 2
                            pending.append(
                                (blk + 1, make_finish(otiles, rr, 2)))
                            for mi, m in enumerate(range(4 * q2,
                                                         4 * q2 + 4)):
                                pending.append(
                                    (blk + 3 + mi, lambda m=m: out_proj(m)))
                            otiles = []
                if pending and pending[0][0] <= b // 2:
                    pending.pop(0)[1]()
                while fi < nfill and (
                        fi < ((b + 2) * nfill) // (total + D)
                        or (deadlines and fi < len(deadlines)
                            and deadlines[fi] <= b // 2)):
                    fillers[fi]()
                    fi += 1

            for _, w in pending:
                w()
            if final:
                return None
            # deferred normalize, emitted into the next pair's stream:
            # DVE-side now (gather den rows to partitions 0-7 via DMA —
            # engine APs cannot start at unaligned partitions, DMA can
            # place anywhere — then one batched reciprocal), PE-side
            # returned as a closure
            dd = rp_pool.tile([8, 512], f32, tag="dd", name="ddt")
            for i, (off, q, o) in enumerate(otiles):
                nc.sync.dma_start(dd[i:i + 1, :], o[64:65, :])
            rr = rp_pool.tile([8, 512], f32, tag="rr", name="rrt")
            with nc.allow_low_precision(reason="softmax denom"):
                nc.vector.reciprocal(rr[:], dd[:])
            return make_finish(otiles, rr, 8)

        # prologue: q chunk 0 + all of k for pair 0 (k columns are consumed
        # across all chunks within the first query quarter); remaining
        # pair-0 q chunks become deadline-paced fillers, v inlined into
        # pair 0, later pairs' projections spread as fillers, and the
        # output projection folds into the final pair per query quarter
        qk_group(0, 0)
        for j in range(NQ4):
            qk_group(4, j)
        finish_prev = None
        for g in range(4):
            fillers, deadlines = [], []
            if g == 0:
                for j in range(1, NQ4):
                    fillers.append(lambda j=j: qk_group(0, j))
                    deadlines.append(8 * j - 3)
            if g < 3:
                for mt in (4 + g + 1, g + 1):
                    for j in range(NQ4):
                        fillers.append(lambda mt=mt, j=j: qk_group(mt, j))
                        deadlines.append(10 ** 9)
            prework = [finish_prev] if finish_prev else []
            finish_prev = attention_pair(g, fillers, deadlines,
                                         inline_v=(g == 0), final=(g == 3),
                                         prework=prework)

    nc.compile()
    return nc


def _in_maps(x, w_qkv, b_qkv, w_out):
    import ml_dtypes
    bf = ml_dtypes.bfloat16
    x = np.asarray(x, np.float32)
    w_qkv = np.asarray(w_qkv, np.float32)
    b_qkv = np.asarray(b_qkv, np.float32)
    w_out = np.asarray(w_out, np.float32)
    maps = []
    for core in range(8):
        b, hg = core // 2, core % 2
        s = slice(hg * 512, hg * 512 + 512)
        maps.append({
            "xT": np.ascontiguousarray(x[b].T).astype(bf),
            "wq": np.ascontiguousarray(w_qkv[:, 0 * C:1 * C][:, s]).astype(bf),
            "wk": np.ascontiguousarray(w_qkv[:, 1 * C:2 * C][:, s]).astype(bf),
            "wv": np.ascontiguousarray(w_qkv[:, 2 * C:3 * C][:, s]).astype(bf),
            "bqk": np.ascontiguousarray(np.concatenate(
                [b_qkv[0 * C:1 * C][s], b_qkv[1 * C:2 * C][s]])
                .reshape(8, P).T),
            "wo": np.ascontiguousarray(w_out[s, :]).astype(bf),
        })
    return maps


def _gather(results, b_qkv, b_out, w_out):
    out = np.zeros((B, N, C), np.float32)
    for core in range(8):
        out[core // 2] += np.asarray(results[core]["out"], np.float32)
    # exact bias terms: softmax rows sum to 1, so +b_v contributes b_v @ w_out
    out += (np.asarray(b_qkv[2 * C:3 * C], np.float32)
            @ np.asarray(w_out, np.float32) + np.asarray(b_out, np.float32))
    return out


def run(x, w_qkv, b_qkv, w_out, b_out, trace=False):
    from concourse.bass_utils import run_bass_kernel_spmd
    if "nc" not in _CACHE:
        _CACHE["nc"] = _build()
    res = run_bass_kernel_spmd(_CACHE["nc"], _in_maps(x, w_qkv, b_qkv, w_out),
                               list(range(8)), trace=trace)
    _CACHE["last_res"] = res
    return _gather(res.results, b_qkv, b_out, w_out), res.exec_time_ns


def kernel(x, w_qkv, b_qkv, w_out, b_out):
    out, _ = run(x, w_qkv, b_qkv, w_out, b_out)
    return out
